# revision 51
# baseline (speedup 1.0000x reference)
"""GroupedQueryAttentionWithRoPE on 8 TRN2 NeuronCores.

Sharding: data-parallel over batch B=2, tensor-parallel over the 16 q heads
(4 heads / core, 2 kv heads / core, Megatron column/row split of the
projections).  Core c handles batch c//4, head group c%4.  Each core returns
its partial out-projection [T, E]; the host sums the 4 TP partials per batch
(bf16 device output, f32 host accumulate) and adds bo.

Device kernel (per core, identical SPMD program):
  - q/k projections in fp8-e4m3 with DoubleRow matmuls (2 contraction
    chunks per instruction, half cycles/row); their quantization error is
    suppressed by the softmax, unlike the v/out paths which stay bf16
  - RoPE in bf16 on DVE (rotate-half swap via partition-shifted copies,
    sin-product on Pool), tables precomputed on the host
  - attention in "S-transposed" orientation: S^T tiles [128 k, 2x512 q]
    from two row-tiled 64-contraction matmuls; exp on ACT (one strided
    instruction per k block, skipping fully-masked columns); the causal
    staircase is a 0/1 mask multiply on Pool on the diagonal blocks
  - softmax denominator comes free as a ones-column appended to v
  - O normalized per-partition (DVE), transposed via PE into O^T, then the
    bf16 out-projection; y staged through SBUF (DVE; ACT where idle) and
    DMA'd out bf16
  - emission order is built by a global scheduler: a queue of S^T/exp
    chunks in stage order plus a FIFO of filler units (PV + normalize,
    transposes, out-projection, next group's projections), each with an
    emission window (earliest position where its inputs exist, deadline
    before its buffers are recycled).  Pacing against the simulated ACT
    backlog keeps the exp stream -- the critical engine at ~81 us busy --
    dense while the in-order PE queue stays fed; input DMAs are sliced by
    q-group so compute starts ~6 us into the kernel.

Local q-head order is [0, 2, 1, 3] (host permutes Wq cols / Wo rows) so the
two row-tiled S^T matmuls read kv0 from SBUF partitions 0-63 and kv1 from
partitions 64-127.
"""

import os

import numpy as np
import ml_dtypes

import concourse.bass as bass
import concourse.mybir as mybir
import concourse.tile as tile
from concourse import bacc
from concourse.bass_utils import run_bass_kernel_spmd

B, T, E = 2, 2048, 1024
N_HEAD, N_KV, HEAD = 16, 8, 64
NCORES, TPD = 8, 4  # 2 (batch) x 4 (head groups)
HQL, HKVL = 4, 2    # local q heads / kv heads per core
KC = E // 128       # contraction chunks
QG = T // 512       # 512-wide q groups
TB = T // 128       # 128-row T blocks

f32 = mybir.dt.float32

_DT_ENV = os.environ.get("BASSK_DT", "bf16")
if _DT_ENV == "f32":
    DT, NP_DT = mybir.dt.float32, np.float32
else:
    DT, NP_DT = mybir.dt.bfloat16, ml_dtypes.bfloat16

# fp8 (DoubleRow) input projections: x, Wq, Wk, Wv quantized to e4m3.
# The error washes out through the softmax / value averaging; the
# out-projection stays bf16 (its error hits y directly).
FP8_PROJ = os.environ.get("BASSK_FP8", "1") == "1"
F8, NP_F8 = mybir.dt.float8e4, ml_dtypes.float8_e4m3


def build_nc(dt=DT):
    """Build the per-core SPMD Bass program (pair-stage pipelined)."""
    from contextlib import ExitStack

    pdt = F8 if FP8_PROJ else dt
    nc = bacc.Bacc(None, target_bir_lowering=False, debug=False)
    with tile.TileContext(nc) as tc, ExitStack() as stk:
        with tc.tile_pool(name="dram", bufs=1, space="DRAM") as dram:
            def din(name, shape, dty):
                return dram.tile(shape, dty, kind="ExternalInput", name=name,
                                 uniquify=False, tag=name)

            xT_d = din("xT", [E, T], dt)
            xT8_d = din("xT8", [E, T], pdt)
            wq_d = din("wq", [E, HQL * HEAD], pdt)
            wk_d = din("wk", [E, HKVL * HEAD], pdt)
            wv_d = din("wv", [E, HKVL * HEAD], dt)
            wo_d = din("wo", [HQL * HEAD, E], dt)
            bq_d = din("bq", [2, 128, 1], f32)
            bk_d = din("bk", [128, 1], f32)
            bvb_d = din("bvb", [128, 128], f32)
            cos_d = din("cosT", [128, T], dt)
            ssin_d = din("ssinT", [128, T], dt)
            idn_d = din("iden", [128, 128], dt)
            msk_d = din("msk", [128, 2 * 128], dt)
            y_d = dram.tile([T, E], dt, kind="ExternalOutput", name="y",
                            uniquify=False, tag="y")

            # ---------------- persistent SBUF ----------------
            const = stk.enter_context(tc.tile_pool(name="const", bufs=1))
            wq_sb = const.tile([128, KC, HQL * HEAD], pdt, tag="wq", name="wq_sb")
            wk_sb = const.tile([128, KC, HKVL * HEAD], pdt, tag="wk", name="wk_sb")
            wv_sb = const.tile([128, KC, HKVL * HEAD], dt, tag="wv", name="wv_sb")
            wo_sb = const.tile([128, 2, E], dt, tag="wo", name="wo_sb")
            idn_sb = const.tile([128, 128], dt, tag="idn", name="idn_sb")
            msk_sb = const.tile([128, 2, 128], dt, tag="msk", name="msk_sb")
            bq_sb = [const.tile([128, 1], f32, tag=f"bq{m}", name=f"bq_sb{m}")
                     for m in range(2)]
            bk_sb = const.tile([128, 1], f32, tag="bk", name="bk_sb")
            bvb_sb = const.tile([128, 128], f32, tag="bvb", name="bvb_sb")
            cos_sb = const.tile([128, T], dt, tag="cos", name="cos_sb")
            ssin_sb = const.tile([128, T], dt, tag="ssin", name="ssin_sb")
            q_dt = [const.tile([128, T], dt, tag=f"qdt{m}", name=f"q_dt{m}")
                    for m in range(2)]
            k_dt = const.tile([128, T], dt, tag="kdt", name="k_dt")
            v_sb = const.tile([128, TB, 2 * (HEAD + 1)], dt, tag="v", name="v_sb")
            ot_sb = [const.tile([128, T], dt, tag=f"ot{m}", name=f"ot_sb{m}")
                     for m in range(2)]
            xT_sb = const.tile([128, KC, T], dt, tag="xT", name="xT_sb")
            xT8_sb = const.tile([128, KC, T], pdt, tag="xT8", name="xT8_sb")

            # transient pools
            rp = stk.enter_context(tc.tile_pool(name="rp", bufs=3))
            pt_pool = stk.enter_context(tc.tile_pool(name="pt", bufs=2))
            sm_pool = stk.enter_context(tc.tile_pool(name="sm", bufs=2))
            ysb_pool = stk.enter_context(tc.tile_pool(name="ysb", bufs=2))
            st_pool = stk.enter_context(tc.tile_pool(name="st", bufs=2, space="PSUM"))
            io_pool = stk.enter_context(tc.tile_pool(name="io", bufs=2, space="PSUM"))
            sml_pool = stk.enter_context(tc.tile_pool(name="sml", bufs=1, space="PSUM"))

            # ---------------- loads (SP queue, earliest-need order) --------
            xv = xT_d.rearrange("(c p) t -> p c t", p=128)
            xv8 = xT8_d.rearrange("(c p) t -> p c t", p=128)

            # V-projection first: its matmuls double as the PE p-state
            # warmup while the K/Q weights and rope tables stream in.
            nc.sync.dma_start(out=wv_sb, in_=wv_d.rearrange("(c p) m -> p c m", p=128))
            nc.sync.dma_start(out=xT_sb[:, 0:4, 0:512], in_=xv[:, 0:4, 0:512])
            nc.sync.dma_start(out=wk_sb, in_=wk_d.rearrange("(c p) m -> p c m", p=128))
            nc.sync.dma_start(out=bvb_sb, in_=bvb_d)
            nc.sync.dma_start(out=xT_sb[:, 4:8, 0:512], in_=xv[:, 4:8, 0:512])
            nc.sync.dma_start(out=xT8_sb[:, :, 0:512], in_=xv8[:, :, 0:512])
            nc.sync.dma_start(out=wq_sb, in_=wq_d.rearrange("(c p) m -> p c m", p=128))
            nc.sync.dma_start(out=bk_sb, in_=bk_d)
            for m in range(2):
                nc.sync.dma_start(out=bq_sb[m], in_=bq_d[m])
            nc.sync.dma_start(out=cos_sb[:, 0:512], in_=cos_d[:, 0:512])
            nc.sync.dma_start(out=ssin_sb[:, 0:512], in_=ssin_d[:, 0:512])
            nc.sync.dma_start(out=xT_sb[:, :, 512:1024], in_=xv[:, :, 512:1024])
            nc.sync.dma_start(out=xT8_sb[:, :, 512:1024], in_=xv8[:, :, 512:1024])
            nc.sync.dma_start(out=msk_sb, in_=msk_d.rearrange("p (h q) -> p h q", h=2))
            nc.sync.dma_start(out=cos_sb[:, 512:T], in_=cos_d[:, 512:T])
            nc.sync.dma_start(out=ssin_sb[:, 512:T], in_=ssin_d[:, 512:T])
            nc.sync.dma_start(out=idn_sb, in_=idn_d)
            nc.sync.dma_start(out=xT_sb[:, :, 1024:1536], in_=xv[:, :, 1024:1536])
            nc.sync.dma_start(out=xT8_sb[:, :, 1024:1536], in_=xv8[:, :, 1024:1536])
            nc.sync.dma_start(out=wo_sb, in_=wo_d.rearrange("(c p) m -> p c m", p=128))
            nc.sync.dma_start(out=xT_sb[:, :, 1536:2048], in_=xv[:, :, 1536:2048])
            nc.sync.dma_start(out=xT8_sb[:, :, 1536:2048], in_=xv8[:, :, 1536:2048])

            # ---------------- emit helpers ----------------
            escale = float(1.0 / np.sqrt(HEAD))

            def rope_to(dst, gs, ps, bias):
                """dst[:, gs] = rope(psum + bias), bf16; swap via partition-
                shifted DVE copies; sin-product on Pool."""
                qr = rp.tile([128, 512], dt, tag="qraw", name="qr")
                nc.vector.tensor_scalar_add(qr, ps, bias)
                sw = rp.tile([128, 512], dt, tag="swp", name="sw")
                for lo, hi in ((0, 32), (64, 96)):
                    nc.vector.tensor_copy(sw[lo:lo + 32, :], qr[hi:hi + 32, :])
                    nc.vector.tensor_copy(sw[hi:hi + 32, :], qr[lo:lo + 32, :])
                t1 = rp.tile([128, 512], dt, tag="t1", name="t1")
                t2 = rp.tile([128, 512], dt, tag="t2", name="t2")
                nc.vector.tensor_mul(t1, qr, cos_sb[:, gs])
                nc.gpsimd.tensor_mul(t2, sw, ssin_sb[:, gs])
                nc.vector.tensor_add(dst[:, gs], t1, t2)

            DR = mybir.MatmulPerfMode.DoubleRow

            def emit_kproj(g):
                gs = slice(g * 512, (g + 1) * 512)
                ps = io_pool.tile([128, 512], f32, tag="io", name="kps")
                if FP8_PROJ:
                    for c in range(0, KC, 2):
                        nc.tensor.matmul(ps, wk_sb[:, c:c + 2, :],
                                         xT8_sb[:, c:c + 2, gs],
                                         start=(c == 0), stop=(c == KC - 2),
                                         perf_mode=DR)
                else:
                    for c in range(KC):
                        nc.tensor.matmul(ps, wk_sb[:, c, :], xT_sb[:, c, gs],
                                         start=(c == 0), stop=(c == KC - 1))
                rope_to(k_dt, gs, ps, bk_sb)

            def emit_qproj(g, m):
                gs = slice(g * 512, (g + 1) * 512)
                ms = slice(m * 128, (m + 1) * 128)
                ps = io_pool.tile([128, 512], f32, tag="io", name="qps")
                if FP8_PROJ:
                    for c in range(0, KC, 2):
                        nc.tensor.matmul(ps, wq_sb[:, c:c + 2, ms],
                                         xT8_sb[:, c:c + 2, gs],
                                         start=(c == 0), stop=(c == KC - 2),
                                         perf_mode=DR)
                else:
                    for c in range(KC):
                        nc.tensor.matmul(ps, wq_sb[:, c, ms], xT_sb[:, c, gs],
                                         start=(c == 0), stop=(c == KC - 1))
                rope_to(q_dt[m], gs, ps, bq_sb[m])

            def emit_vproj(g, tb):
                ts_ = slice(tb * 128, (tb + 1) * 128)
                ps = io_pool.tile([128, 512], f32, tag="io", name="vps")
                for c in range(KC):
                    nc.tensor.matmul(ps[:, 0:128], xT_sb[:, c, ts_],
                                     wv_sb[:, c, :],
                                     start=(c == 0), stop=(c == KC - 1))
                nc.vector.tensor_add(
                    v_sb[:, tb, :].rearrange("p (h e) -> p h e", h=2)[:, :, 0:HEAD],
                    ps[:, 0:128].rearrange("p (h d) -> p h d", h=2),
                    bvb_sb.rearrange("p (h d) -> p h d", h=2))

            def emit_vones(g):
                nc.gpsimd.memset(v_sb[:, 4 * g:4 * g + 4, HEAD:HEAD + 1], 1.0)
                nc.gpsimd.memset(
                    v_sb[:, 4 * g:4 * g + 4, 2 * HEAD + 1:2 * HEAD + 2], 1.0)

            # pts[pair][kb] SBUF tiles for the current group, tag-rotated
            pts = [[None] * (4 * QG) for _ in range(2)]

            def emit_S(g, pair, kb):
                """S^T + exp (+ causal mask) for one 128-wide k block."""
                ks = slice(kb * 128, (kb + 1) * 128)
                j = kb - 4 * g  # >= 0 on the causal staircase
                off = max(j, 0) * 128  # q cols < off are fully masked
                st = st_pool.tile([128, 1024], f32, tag="st", name="st")
                for hi in range(2):
                    hp = slice(hi * 64, hi * 64 + 64)
                    nc.tensor.matmul(
                        st[:, hi * 512 + off:(hi + 1) * 512],
                        k_dt[hp, ks],
                        q_dt[pair][hp, g * 512 + off:(g + 1) * 512],
                        start=True, stop=True)
                pt = pt_pool.tile([128, 1024], dt, tag=f"pt{kb}",
                                  name=f"pt{kb}", bufs=2)
                if off == 0:
                    nc.scalar.activation(pt, st,
                                         mybir.ActivationFunctionType.Exp,
                                         scale=escale)
                else:
                    stv = st.rearrange("p (h q) -> p h q", h=2)[:, :, off:512]
                    ptv = pt.rearrange("p (h q) -> p h q", h=2)[:, :, off:512]
                    nc.scalar.activation(ptv, stv,
                                         mybir.ActivationFunctionType.Exp,
                                         scale=escale)
                if j >= 0:
                    # zero the upper triangle of the two diagonal 128-col
                    # blocks with a 0/1 mask multiply (split DVE/Pool)
                    dg = pt.rearrange("p (h q) -> p h q", h=2)[
                        :, :, j * 128:(j + 1) * 128]
                    nc.gpsimd.tensor_mul(dg, dg, msk_sb)
                pts[pair][kb] = pt

            def emit_PVa(g, pair, qb, state):
                """P@V accumulation + normalization for one 128-row q block."""
                onrm = []
                for hi in range(2):
                    oacc = sml_pool.tile([128, HEAD + 1], f32, tag="sml",
                                         name="oacc", bufs=2)
                    nq = 4 * g + qb + 1
                    for kb in range(nq):
                        nc.tensor.matmul(
                            oacc,
                            pts[pair][kb][:, hi * 512 + qb * 128:
                                          hi * 512 + (qb + 1) * 128],
                            v_sb[:, kb, hi * 65:hi * 65 + 65],
                            start=(kb == 0), stop=(kb == nq - 1))
                    rden = sm_pool.tile([128, 1], f32, tag=f"rden{hi}",
                                        name=f"rden{hi}", bufs=2)
                    nc.vector.reciprocal(rden, oacc[:, HEAD:HEAD + 1])
                    on = sm_pool.tile([128, HEAD], dt, tag=f"onrm{hi}",
                                      name=f"onrm{hi}", bufs=2)
                    nc.vector.tensor_scalar_mul(on, oacc[:, 0:HEAD], rden)
                    onrm.append(on)
                state[qb] = onrm

            def emit_PVb(g, pair, qb, state):
                """transpose O -> O^T columns of ot_sb."""
                onrm = state[qb]
                tp = sml_pool.tile([128, 128], dt, tag="sml", name="tp",
                                   bufs=2)
                nc.tensor.transpose(tp[0:64, :], onrm[0], idn_sb)
                nc.tensor.transpose(tp[64:128, :], onrm[1], idn_sb,
                                    tile_position=(0, 64))
                qcol = slice((g * 4 + qb) * 128, (g * 4 + qb + 1) * 128)
                nc.vector.tensor_copy(ot_sb[pair][:, qcol], tp)

            def emit_outproj(g, qb):
                rs = slice((g * 4 + qb) * 128, (g * 4 + qb + 1) * 128)
                ysb = ysb_pool.tile([128, 2, 512], dt, tag="ysb", name="ysb")
                for nh in range(2):
                    ns = slice(nh * 512, (nh + 1) * 512)
                    yp = io_pool.tile([128, 512], f32, tag="io", name="yp")
                    nc.tensor.matmul(yp, ot_sb[0][:, rs], wo_sb[:, 0, ns],
                                     start=True, stop=False)
                    nc.tensor.matmul(yp, ot_sb[1][:, rs], wo_sb[:, 1, ns],
                                     start=False, stop=True)
                    if g == 0 or (g == QG - 1 and (nh == 0 or qb >= 2)):
                        # ACT is idle during the tail; stage half the final
                        # group's y through it instead of DVE
                        nc.scalar.activation(ysb[:, nh, :], yp,
                                             mybir.ActivationFunctionType.Copy)
                    else:
                        nc.vector.tensor_copy(ysb[:, nh, :], yp)
                nc.sync.dma_start(out=y_d[rs, :], in_=ysb)

            # ------------- globally-paced emission -------------
            # One global queue of S chunks (in stage order) plus a FIFO of
            # filler units, each with an emission window [mn, mx): legal
            # after S chunk mn-1 has been emitted (data it reads exists),
            # required before S chunk mx is emitted (buffers it reads get
            # recycled / ranges it writes get read).  Emission is paced by
            # the simulated ACT backlog so the exp stream never starves
            # while the in-order PE queue stays fed.
            PE_CYC = 1.0 / 2.4

            def s_cost(g, kb):
                off = max(kb - 4 * g, 0) * 128
                cols = 2 * (512 - off)
                return (cols * PE_CYC, cols * 0.833 + 185.0)  # (pe, act)

            def pv_parts(g, pair):
                state = {}

                def a(qb):
                    return (lambda qb=qb: emit_PVa(g, pair, qb, state),
                            (4 * g + qb + 1) * 2 * 65 * PE_CYC)

                def bqb(qb):
                    return (lambda qb=qb: emit_PVb(g, pair, qb, state),
                            2 * 128 * PE_CYC)

                def op(qb):
                    return (lambda qb=qb: emit_outproj(g, qb),
                            4 * 512 * PE_CYC)

                return a, bqb, op

            def proj_units(g):
                u = [(lambda g=g: emit_kproj(g), 4 * 512 * PE_CYC),
                     (lambda g=g: emit_qproj(g, 0), 4 * 512 * PE_CYC),
                     (lambda g=g: emit_qproj(g, 1), 4 * 512 * PE_CYC)]
                u += [(lambda g=g, tb=tb: emit_vproj(g, tb), 8 * 128 * PE_CYC)
                      for tb in range(4 * g, 4 * g + 4)]
                u += [(lambda g=g: emit_vones(g), 0.0)]
                return u

            # ---------------- prologue (V first: PE p-state warmup) -------
            for tb in range(4):
                emit_vproj(0, tb)
            emit_vones(0)
            emit_kproj(0)
            emit_qproj(0, 0)
            emit_qproj(0, 1)

            # ---------------- global queues ----------------
            S_list = []
            S_idx = {}
            for g in range(QG):
                for pair in range(2):
                    for kb in range(4 * g + 4):
                        S_idx[(g, pair, kb)] = len(S_list)
                        pe_c, act_c = s_cost(g, kb)
                        S_list.append((lambda g=g, pair=pair, kb=kb:
                                       emit_S(g, pair, kb), pe_c, act_c))
            NS = len(S_list)

            F_list = []  # (fn, pe_cost, mn, mx)

            def add_f(unit, mn, mx):
                F_list.append((unit[0], unit[1], mn, mx))

            for g in range(QG):
                for pair in range(2):
                    a, bqb, op = pv_parts(g, pair)
                    mx = S_idx[(g + 1, pair, 0)] if g + 1 < QG else NS + 1
                    for qb in range(4):
                        mn = S_idx[(g, pair, 4 * g + qb)] + 1
                        add_f(a(qb), mn, mx)
                        add_f(bqb(qb), mn, mx)
                        if pair == 1:
                            add_f(op(qb), mn, NS + 1)
                    if pair == 0 and g + 1 < QG:
                        # projections of the next group between the two
                        # pairs' PV blocks; K/Q are deadlined early in the
                        # p1 stage so their rope chains complete well before
                        # the next group's S chunks need q_dt/k_dt
                        mn = S_idx[(g, 1, 0)] + 1
                        pu = proj_units(g + 1)
                        add_f(pu[0], mn, S_idx[(g + 1, 0, 4 * (g + 1))])
                        add_f(pu[1], mn, S_idx[(g + 1, 0, 0)])
                        add_f(pu[2], mn, S_idx[(g + 1, 1, 0)])
                        for vu in pu[3:]:
                            add_f(vu, mn, S_idx[(g + 1, 0, 4 * (g + 1))])

            # ---------------- emission loop ----------------
            # sufmin_mx[j] = earliest deadline among F_list[j:] so a near
            # deadline deeper in the FIFO forces the units ahead of it out
            sufmin_mx = [0] * (len(F_list) + 1)
            sufmin_mx[len(F_list)] = NS + 2
            for j in range(len(F_list) - 1, -1, -1):
                sufmin_mx[j] = min(F_list[j][3], sufmin_mx[j + 1])

            act_lead = float(os.environ.get("BASSK_LEAD", "600"))
            lead_lo = float(os.environ.get("BASSK_LEAD_LO", "-1e18"))
            lead_hi = float(os.environ.get("BASSK_LEAD_HI", "1e18"))
            si = fi = 0
            while si < NS or fi < len(F_list):
                if fi < len(F_list):
                    fn, f_pe, mn, mx = F_list[fi]
                    if sufmin_mx[fi] <= si or si >= NS:
                        fn()
                        act_lead -= f_pe
                        fi += 1
                        continue
                    if act_lead > 0 and si >= mn:
                        fn()
                        act_lead -= f_pe
                        fi += 1
                        continue
                s_fn, pe_c, act_c = S_list[si]
                s_fn()
                act_lead = min(max(act_lead + act_c - pe_c, lead_lo), lead_hi)
                si += 1

    nc.finalize()
    return nc


# local head order in the chunks: chunk0 = heads (0, 2), chunk1 = heads (1, 3)
_HEAD_ORDER = [0, 2, 1, 3]


def _rope_tables_np():
    inv_freq = (1.0 / (10000.0 ** (np.arange(0, HEAD, 2, dtype=np.float32) / HEAD))
                ).astype(np.float32)                       # [32]
    ang = np.arange(T, dtype=np.float32)[:, None] * inv_freq[None, :]  # [T, 32]
    sin, cos = np.sin(ang), np.cos(ang)                    # f32 [T, 32]
    idx = np.arange(HEAD) % 32
    cos_d = cos[:, idx].T                                  # [64, T]
    sin_d = sin[:, idx].T
    sign = np.where(np.arange(HEAD) < 32, -1.0, 1.0).astype(np.float32)
    ssin_d = sin_d * sign[:, None]
    cosT = np.tile(cos_d, (2, 1)).astype(np.float32)       # [128, T]
    ssinT = np.tile(ssin_d, (2, 1)).astype(np.float32)
    return np.ascontiguousarray(cosT), np.ascontiguousarray(ssinT)


def make_in_maps(x, Wq, bq, Wk, bk, Wv, bv, Wo):
    x = np.asarray(x, np.float32)
    cosT, ssinT = _rope_tables_np()
    iden = np.eye(128, dtype=np.float32).astype(NP_DT)
    kk = np.arange(128)[:, None]
    qq = np.arange(128)[None, :]
    m1 = (kk <= qq).astype(np.float32)                     # [128, 128]
    msk = np.tile(m1, (1, 2)).astype(NP_DT)                # [128, 256]
    in_maps = []
    for c in range(NCORES):
        b, tp = c // TPD, c % TPD
        heads = [4 * tp + h for h in _HEAD_ORDER]
        wq_p = np.concatenate([Wq[:, h * 64:(h + 1) * 64] for h in heads], axis=1)
        bq_p = np.concatenate([bq[h * 64:(h + 1) * 64] for h in heads])
        wo_p = np.concatenate([Wo[h * 64:(h + 1) * 64, :] for h in heads], axis=0)
        kv = slice(tp * 128, (tp + 1) * 128)
        np_p = NP_F8 if FP8_PROJ else NP_DT
        xTb = np.ascontiguousarray(x[b].T)
        in_maps.append({
            "xT": xTb.astype(NP_DT),
            "xT8": xTb.astype(np_p),
            "wq": np.ascontiguousarray(wq_p).astype(np_p),
            "wk": np.ascontiguousarray(Wk[:, kv]).astype(np_p),
            "wv": np.ascontiguousarray(Wv[:, kv]).astype(NP_DT),
            "wo": np.ascontiguousarray(wo_p).astype(NP_DT),
            "bq": np.ascontiguousarray(bq_p, np.float32).reshape(2, 128, 1),
            "bk": np.ascontiguousarray(bk[kv], np.float32).reshape(128, 1),
            "bvb": np.tile(np.asarray(bv[kv], np.float32)[None, :], (128, 1)),
            "cosT": cosT.astype(NP_DT),
            "ssinT": ssinT.astype(NP_DT),
            "iden": iden,
            "msk": msk,
        })
    return in_maps


_NC_CACHE = {}


def _get_nc():
    if DT not in _NC_CACHE:
        _NC_CACHE[DT] = build_nc(DT)
    return _NC_CACHE[DT]


def kernel(x, Wq, bq, Wk, bk, Wv, bv, Wo, bo):
    nc = _get_nc()
    in_maps = make_in_maps(x, Wq, bq, Wk, bk, Wv, bv, Wo)
    res = run_bass_kernel_spmd(nc, in_maps, list(range(NCORES)))
    out = np.zeros((B, T, E), np.float32)
    for c in range(NCORES):
        out[c // TPD] += np.asarray(res.results[c]["y"], np.float32)
    out += np.asarray(bo, np.float32)[None, None, :]
    return out
